# revision 12
# baseline (speedup 1.0000x reference)
"""Multi-head attention + residual + batchnorm on 8 trn2 NeuronCores.

Sharding: core c owns head h = c for ALL 4 batches. Head h covers output
features [h*128, (h+1)*128), so batchnorm statistics over (batch, seq)
are fully local to the core: no cross-core collective at all.

All device compute is feature-major so every matmul contracts over the
partition dim with zero on-chip transposes:

  QT[u,t] = Wq_h @ query[b].T     fp8 DoubleRow (K=256/pass), descaled
  KT[u,t] = Wk_h @ keys[b].T      fp8 DoubleRow from on-chip-cast keys
  V[t,u]  = keys[b] @ Wv_h.T      bf16 (N=128 matmuls, FWL), stored fp8
  ST[k,q] = KT.T-contract QT      bf16 (K=128: DoubleRow not applicable)
  PT      = exp(ST)               ACT, PSUM->SBUF, fp8 (scores bounded)
  OT[u,q] = sum_k V[k,u]*PT[k,q]  fp8 DoubleRow
  r[q]    = sum_k PT[k,q]         fp8 DoubleRow ones-matmuls (f32 acc)
  o_res   = OT/r + query[b].T     f32 residual
  batchnorm over (b,s): bn_stats per chunk, local bn_aggr, affine.

fp8 scaling: weights are scaled x32 on host (unit std, fits e4m3);
the 1/32 score scale plus the x32x32 weight descale is folded into the
QT/KT PSUM copy-out factor sq = sk = 1/sqrt(32768).

DMA strategy (per-queue throughput is descriptor-rate-bound, ~23
descriptors/us): keys ship once as bf16 (the K-projection's fp8 copy
is cast on-chip by the DVE), and the host layouts put [P] outermost so
every DMA moves 4-16KB per partition in one descriptor. Streams are
spread over the gpsimd (keys), sync (query-fp8), and scalar (residual
+ weights) queues.

The tensor-engine emission interleaves, between the score matmuls of
chunk N, the AV+rowsum drain of chunk N-1 plus projection work for
upcoming batches, so the PE never stalls on the ACT engine (exp is the
per-chunk ACT straggler) freeing score PSUM banks, and the HAM clock
gate stays at 8/8.
"""
import sys

sys.path.insert(0, "/opt/trn_rl_repo")

from collections import deque

import numpy as np

import concourse.bass as bass
import concourse.tile as tile
from concourse import bacc, mybir
from concourse.bass_utils import run_bass_kernel_spmd

F32 = mybir.dt.float32
BF16 = mybir.dt.bfloat16
FP8 = mybir.dt.float8e4
AF = mybir.ActivationFunctionType
PM_DR = mybir.MatmulPerfMode.DoubleRow
ALU = mybir.AluOpType
NPBF16 = mybir.dt.np(BF16)
NPFP8 = mybir.dt.np(FP8)

B, S, D, H = 4, 2048, 1024, 8
DH = 128
P = 128
TC = 4                # 512-token chunks per sequence
TCW = 512
DT = 8                # 128-wide d-tiles in D
DP = 4                # d-tile pairs (DoubleRow K=256)
KT_N = 16             # 128-wide k-tiles per sequence
EPS = 1e-5
WSCALE = 32.0
SQK = 1.0 / np.sqrt(32768.0)   # QT/KT copy-out descale; sq*sk*D = 1/32


def _build():
    nc = bacc.Bacc(num_swdge_queues=1)
    qt8 = nc.declare_dram_parameter(
        "qt8", [B, P, TC, DP, 2, TCW], FP8, isOutput=False)
    kt16 = nc.declare_dram_parameter(
        "kt16", [B, P, TC, DT, TCW], BF16, isOutput=False)
    wq8 = nc.declare_dram_parameter("wq8", [P, DP, 2, DH], FP8, isOutput=False)
    wk8 = nc.declare_dram_parameter("wk8", [P, DP, 2, DH], FP8, isOutput=False)
    wv16 = nc.declare_dram_parameter("wv16", [P, DT, DH], BF16, isOutput=False)
    qres = nc.declare_dram_parameter("qres", [B, P, S], F32, isOutput=False)
    gamma = nc.declare_dram_parameter("gamma", [P, 1], F32, isOutput=False)
    beta = nc.declare_dram_parameter("beta", [P, 1], F32, isOutput=False)
    out = nc.declare_dram_parameter("out", [P, B, S], F32, isOutput=True)

    with tile.TileContext(nc) as tc:
        with (
            tc.tile_pool(name="persist", bufs=1) as persist,
            tc.tile_pool(name="xq8", bufs=2) as xq8p,       # per-batch tiles
            tc.tile_pool(name="xk16", bufs=3) as xk16p,     # half-batch tiles
            tc.tile_pool(name="kf8", bufs=6) as kf8p,       # cast scratch
            tc.tile_pool(name="pt", bufs=2) as ptp,
            tc.tile_pool(name="rb", bufs=2) as rbp,
            tc.tile_pool(name="otmp", bufs=2) as otmpp,
            tc.tile_pool(name="ppsum", bufs=2, space="PSUM") as ppsum,
            tc.tile_pool(name="spsum", bufs=2, space="PSUM") as spsum,
            tc.tile_pool(name="opsum", bufs=2, space="PSUM") as opsum,
        ):
            # ---- persistent SBUF ----
            QT = persist.tile([P, B, S], BF16)            # (dh, b, q) 16KB/p
            KT = persist.tile([P, B, KT_N, P], BF16)      # (dh, b, kt, k) 16KB/p
            V8 = persist.tile([P, B, KT_N, DH], FP8)      # (t128, b, kt, u) 8KB/p
            o_res = persist.tile([P, B, S], F32)          # 32KB/p
            bstat = persist.tile([P, B * TC, nc.vector.BN_STATS_DIM], F32)
            wq_s = persist.tile([P, DP, 2, DH], FP8)
            wk_s = persist.tile([P, DP, 2, DH], FP8)
            wv_s = persist.tile([P, DT, DH], BF16)
            gam = persist.tile([P, 1], F32)
            bet = persist.tile([P, 1], F32)
            ones_b = persist.tile([P, P], BF16)
            ones8 = persist.tile([P, 2, P], FP8)
            eps_t = persist.tile([P, 1], F32)
            warm = persist.tile([P, 1], F32)
            mv = persist.tile([P, 2], F32)
            stdt = persist.tile([P, 1], F32)
            rstd = persist.tile([P, 1], F32)
            scl = persist.tile([P, 1], F32)
            shf = persist.tile([P, 1], F32)

            # ---- preamble ----
            nc.vector.memset(eps_t[:], float(EPS))
            nc.vector.memset(ones_b[:], 1.0)
            nc.vector.memset(ones8[:], 1.0)

            xtiles = {}   # streamed tiles keyed by (kind, b[, tc])

            # batch-0 keys stream first (K-proj gates everything),
            # spread across all three DMA queues
            qeng = [nc.gpsimd, nc.sync, nc.scalar, nc.gpsimd]
            nc.scalar.dma_start(wk_s[:], wk8[:])
            for h in range(2):
                t = xk16p.tile([P, 2, DT, TCW], BF16, tag="xk16", name="t")
                for j in range(2):
                    tci = 2 * h + j
                    qeng[tci].dma_start(t[:, j], kt16[0, :, tci])
                    xtiles[("k16", 0, tci)] = (t, j)
            nc.scalar.dma_start(wq_s[:], wq8[:])
            t0q = xq8p.tile([P, TC, DP, 2, TCW], FP8, tag="xq", name="t0q")
            for tci in range(TC):
                nc.sync.dma_start(t0q[:, tci], qt8[0, :, tci])
            xtiles[("q8", 0)] = t0q
            nc.scalar.dma_start(wv_s[:], wv16[:])
            nc.sync.dma_start(o_res[:, 0, :], qres[0])
            nc.scalar.dma_start(gam[:], gamma[:])
            nc.scalar.dma_start(bet[:], beta[:])
            nc.scalar.activation(out=warm[:], in_=eps_t[:], func=AF.Exp)

            # PE warmup: pull the HAM clock gate to 8/8 before real work
            wps = ppsum.tile([P, TCW], F32, tag="pp", name="wps")
            for _ in range(20):
                nc.tensor.matmul(
                    wps[:, 0:P], ones_b[:], ones_b[:],
                    start=True, stop=True, skip_group_check=True,
                )

            def dma_batch(b):
                for h in range(2):
                    t = xk16p.tile([P, 2, DT, TCW], BF16, tag="xk16")
                    qeng[(b + h) % 3].dma_start(
                        t[:], kt16[b, :, bass.ts(h, 2)]
                    )
                    xtiles[("k16", b, 2 * h)] = (t, 0)
                    xtiles[("k16", b, 2 * h + 1)] = (t, 1)
                t = xq8p.tile([P, TC, DP, 2, TCW], FP8, tag="xq")
                nc.sync.dma_start(t[:], qt8[b])
                xtiles[("q8", b)] = t
                nc.scalar.dma_start(o_res[:, b, :], qres[b])

            # ---- tensor-work units (each ~0.3-0.7us of PE time) ----
            def kproj_units(b):
                units = []
                for tci in range(TC):
                    ps = [None]

                    def u1(b=b, tci=tci, ps=ps):
                        ps[0] = ppsum.tile([P, TC, P], F32, tag="pp", name="pk")
                        xt, hj = xtiles[("k16", b, tci)]
                        for dp in range(2):
                            kf = kf8p.tile([P, 2, TCW], FP8, tag="kf", name="kf")
                            nc.vector.tensor_copy(
                                kf[:], xt[:, hj, bass.ts(dp, 2), :]
                            )
                            nc.tensor.matmul(
                                ps[0][:], wk_s[:, dp], kf[:],
                                start=(dp == 0), stop=False,
                                perf_mode=PM_DR, skip_group_check=True,
                            )

                    def u2(b=b, tci=tci, ps=ps):
                        xt, hj = xtiles[("k16", b, tci)]
                        for dp in range(2, DP):
                            kf = kf8p.tile([P, 2, TCW], FP8, tag="kf", name="kf")
                            nc.vector.tensor_copy(
                                kf[:], xt[:, hj, bass.ts(dp, 2), :]
                            )
                            nc.tensor.matmul(
                                ps[0][:], wk_s[:, dp], kf[:],
                                start=False, stop=(dp == DP - 1),
                                perf_mode=PM_DR, skip_group_check=True,
                            )
                        nc.vector.tensor_scalar(
                            KT[:, b, bass.ts(tci, TC), :], ps[0][:],
                            float(SQK), None, ALU.mult,
                        )

                    units += [u1, u2]
                return units

            def qproj_units(b, tci):
                ps = [None]

                def u1(b=b, tci=tci, ps=ps):
                    ps[0] = ppsum.tile([P, TCW], F32, tag="pp", name="pq")
                    xt = xtiles[("q8", b)]
                    for dp in range(2):
                        nc.tensor.matmul(
                            ps[0][:], wq_s[:, dp], xt[:, tci, dp],
                            start=(dp == 0), stop=False,
                            perf_mode=PM_DR, skip_group_check=True,
                        )

                def u2(b=b, tci=tci, ps=ps):
                    xt = xtiles[("q8", b)]
                    for dp in range(2, DP):
                        nc.tensor.matmul(
                            ps[0][:], wq_s[:, dp], xt[:, tci, dp],
                            start=False, stop=(dp == DP - 1),
                            perf_mode=PM_DR, skip_group_check=True,
                        )
                    nc.vector.tensor_scalar(
                        QT[:, b, bass.ts(tci, TCW)], ps[0][:],
                        float(SQK), None, ALU.mult,
                    )

                return [u1, u2]

            def vproj_units(b):
                units = []
                for tci in range(TC):
                    ps = [None]
                    for sub in range(4):

                        def u(b=b, tci=tci, sub=sub, ps=ps):
                            if sub == 0:
                                ps[0] = ppsum.tile([P, 4, DH], F32, tag="pp",
                                                   name="pv")
                            xt, hj = xtiles[("k16", b, tci)]
                            for d in range(DT):
                                nc.tensor.matmul(
                                    ps[0][:, sub, :],
                                    xt[:, hj, d, bass.ts(sub, P)],
                                    wv_s[:, d, :],
                                    start=(d == 0), stop=(d == DT - 1),
                                    skip_group_check=True,
                                )
                            if sub == 3:
                                nc.vector.tensor_copy(
                                    V8[:, b, bass.ts(tci, 4), :], ps[0][:]
                                )

                        units.append(u)
                return units

            pending = {}       # key -> deque of unit callables
            order = deque()    # key pop order
            drain_q = deque()
            late_q = deque()

            def push(key, units):
                pending[key] = deque(units)
                order.append(key)

            def flush(key):
                q = pending.get(key)
                while q:
                    q.popleft()()

            def pop_fill(n):
                for _ in range(n):
                    if drain_q:
                        drain_q.popleft()()
                        continue
                    while order and not pending.get(order[0]):
                        order.popleft()
                    if order:
                        pending[order[0]].popleft()()

            prev = {}

            def make_drain(b, q_i, PT, ps_o, ps_r):
                """AV + rowsum of chunk (b, q_i): 4 units x (2+2) DR MMs."""
                units = []
                for g in range(4):

                    def uav(g=g, b=b, PT=PT, ps_o=ps_o, ps_r=ps_r):
                        for kp in (2 * g, 2 * g + 1):
                            nc.tensor.matmul(
                                ps_o[:],
                                V8[:, b, bass.ts(kp, 2), :],
                                PT[:, bass.ts(kp, 2), :],
                                start=(kp == 0), stop=(kp == KT_N // 2 - 1),
                                perf_mode=PM_DR, skip_group_check=True,
                            )
                            nc.tensor.matmul(
                                ps_r[:],
                                ones8[:],
                                PT[:, bass.ts(kp, 2), :],
                                start=(kp == 0), stop=(kp == KT_N // 2 - 1),
                                perf_mode=PM_DR, skip_group_check=True,
                            )

                    units.append(uav)
                return units

            def make_fin(b, q_i, ps_o, ps_r):
                """1/r + attention normalize + residual add + bn_stats."""

                def ufin(b=b, q_i=q_i, ps_o=ps_o, ps_r=ps_r):
                    rb = rbp.tile([P, TCW], F32, tag="rb")
                    nc.vector.reciprocal_approx_fast(out=rb[:], in_=ps_r[:])
                    otmp = otmpp.tile([P, TCW], F32, tag="ot")
                    nc.vector.tensor_tensor(otmp[:], ps_o[:], rb[:], ALU.mult)
                    dst = o_res[:, b, bass.ts(q_i, TCW)]
                    nc.vector.tensor_add(dst, dst, otmp[:])
                    nc.vector.bn_stats(out=bstat[:, b * TC + q_i, :], in_=dst)

                return ufin

            def emit_chunk(b, q_i):
                # correctness fences: everything this chunk's matmuls read
                # must already be emitted (program order defines deps)
                flush(("k", b))
                flush(("q", b, q_i))
                PT = ptp.tile([P, KT_N, TCW], FP8, tag="pt")
                if prev:
                    flush(("v", prev["b"]))   # AV drain needs V tiles
                    drain_q.extend(
                        make_drain(prev["b"], prev["q_i"], prev["PT"],
                                   prev["ps_o"], prev["ps_r"])
                    )
                    late_q.append(
                        make_fin(prev["b"], prev["q_i"], prev["ps_o"],
                                 prev["ps_r"])
                    )
                ps_o = opsum.tile([P, TCW], F32, tag="op", name="ps_o")
                ps_r = opsum.tile([P, TCW], F32, tag="op", name="ps_r")
                for kp in range(KT_N // 2):
                    ps_s = spsum.tile([P, 2, TCW], F32, tag="sp")
                    for j in range(2):
                        nc.tensor.matmul(
                            ps_s[:, j, :],
                            KT[:, b, 2 * kp + j, :],
                            QT[:, b, bass.ts(q_i, TCW)],
                            start=True, stop=True, skip_group_check=True,
                        )
                    nc.scalar.activation(
                        out=PT[:, bass.ts(kp, 2), :], in_=ps_s[:], func=AF.Exp
                    )
                    if kp == 6 and late_q:
                        late_q.popleft()()
                        pop_fill(1)
                    else:
                        pop_fill(2)
                prev.clear()
                prev.update({"b": b, "q_i": q_i, "PT": PT, "ps_o": ps_o,
                             "ps_r": ps_r})

            # ---- emission ----
            for u in kproj_units(0):
                u()
            for u in qproj_units(0, 0):
                u()

            for b in range(B):
                for q_i in range(TC):
                    if b == 0 and q_i == 0:
                        push(("v", 0), vproj_units(0))
                    if q_i == 0 and b < B - 1:
                        dma_batch(b + 1)
                    if q_i == 1 and b < B - 1:
                        push(("k", b + 1), kproj_units(b + 1))
                    if q_i == 2 and b < B - 1:
                        push(("q", b + 1, 0), qproj_units(b + 1, 0))
                        push(("v", b + 1), vproj_units(b + 1))
                    if q_i < TC - 1:
                        push(("q", b, q_i + 1), qproj_units(b, q_i + 1))
                    emit_chunk(b, q_i)

            # drain the last chunk + any remaining stragglers
            flush(("v", prev["b"]))
            drain_q.extend(
                make_drain(prev["b"], prev["q_i"], prev["PT"], prev["ps_o"],
                           prev["ps_r"])
            )
            late_q.append(
                make_fin(prev["b"], prev["q_i"], prev["ps_o"], prev["ps_r"])
            )
            nc.scalar.activation(out=warm[:], in_=eps_t[:], func=AF.Sqrt)
            while drain_q or any(pending.get(k) for k in list(order)):
                pop_fill(1)
            while late_q:
                late_q.popleft()()

            # ---- batchnorm finale (fully local) ----
            nc.vector.bn_aggr(out=mv[:], in_=bstat[:])
            nc.scalar.activation(
                out=stdt[:], in_=mv[:, 1:2], func=AF.Sqrt, bias=eps_t[:]
            )
            nc.vector.reciprocal(out=rstd[:], in_=stdt[:])
            nc.vector.tensor_mul(scl[:], gam[:], rstd[:])
            nc.vector.tensor_mul(shf[:], mv[:, 0:1], scl[:])
            nc.vector.tensor_sub(shf[:], bet[:], shf[:])
            for b in range(B):
                for half in range(2):
                    sl = bass.ts(half, S // 2)
                    src = o_res[:, b, sl]
                    if half == 0:
                        nc.vector.tensor_scalar(
                            src, src, scl[:], shf[:], ALU.mult, ALU.add
                        )
                    else:
                        nc.scalar.activation(
                            out=src, in_=src, func=AF.Identity,
                            bias=shf[:], scale=scl[:],
                        )
                if b == 1:
                    nc.sync.dma_start(out[:, 0:2, :], o_res[:, 0:2, :])
                elif b == 3:
                    nc.scalar.dma_start(out[:, 2:4, :], o_res[:, 2:4, :])

    nc.finalize()
    return nc


_NC = None


def _get_nc():
    global _NC
    if _NC is None:
        _NC = _build()
    return _NC


def _make_in_maps(query, keys, Wq, Wk, Wv, gamma, beta):
    query = np.asarray(query, dtype=np.float32)
    keys = np.asarray(keys, dtype=np.float32)
    Wq = np.asarray(Wq, dtype=np.float32)
    Wk = np.asarray(Wk, dtype=np.float32)
    Wv = np.asarray(Wv, dtype=np.float32)
    gamma = np.asarray(gamma, dtype=np.float32)
    beta = np.asarray(beta, dtype=np.float32)

    qT = np.ascontiguousarray(query.transpose(0, 2, 1))   # (B, D, S)
    kT = np.ascontiguousarray(keys.transpose(0, 2, 1))

    # (B, D, S) -> [B, P, TC, DP, 2, TCW] fp8 (P outermost per batch)
    v = qT.reshape(B, DP, 2, P, TC, TCW).transpose(0, 3, 4, 1, 2, 5)
    qt8 = np.ascontiguousarray(v.astype(NPFP8))

    # (B, D, S) -> [B, P, TC, DT, TCW] bf16
    v = kT.reshape(B, DT, P, TC, TCW).transpose(0, 2, 3, 1, 4)
    kt16 = np.ascontiguousarray(v.astype(NPBF16))

    in_maps = []
    for c in range(8):
        rows = slice(DH * c, DH * (c + 1))

        def packw8(w):  # rows of W -> [P, DP, 2, DH] fp8, scaled x32
            wt = np.ascontiguousarray(w[rows].T * WSCALE)   # (D, 128)
            v = wt.reshape(DP, 2, P, DH).transpose(2, 0, 1, 3)
            return np.ascontiguousarray(v.astype(NPFP8))

        wv_t = np.ascontiguousarray(Wv[rows].T)             # (D, 128)
        in_maps.append(
            {
                "qt8": qt8,
                "kt16": kt16,
                "wq8": packw8(Wq),
                "wk8": packw8(Wk),
                "wv16": np.ascontiguousarray(
                    wv_t.reshape(DT, P, DH).transpose(1, 0, 2).astype(NPBF16)
                ),
                "qres": np.ascontiguousarray(
                    query[:, :, rows].transpose(0, 2, 1)
                ),  # (B, 128, S)
                "gamma": np.ascontiguousarray(gamma[rows].reshape(P, 1)),
                "beta": np.ascontiguousarray(beta[rows].reshape(P, 1)),
            }
        )
    return in_maps


def _run(in_maps, trace=False, **kw):
    nc = _get_nc()
    return run_bass_kernel_spmd(
        nc, in_maps, core_ids=list(range(8)), trace=trace, **kw
    )


def kernel(query, keys, Wq, Wk, Wv, gamma, beta):
    in_maps = _make_in_maps(query, keys, Wq, Wk, Wv, gamma, beta)
    res = _run(in_maps)
    output = np.empty((B, S, D), dtype=np.float32)
    for c in range(8):
        oc = res.results[c]["out"]                    # (128, B, S)
        output[:, :, DH * c : DH * (c + 1)] = oc.transpose(1, 2, 0)
    return output


# revision 15
# speedup vs baseline: 1.0218x; 1.0218x over previous
"""Multi-head attention + residual + batchnorm on 8 trn2 NeuronCores.
Measured: 238974 ns (baseline two-phase (b,head-group) kernel: 303878).

Sharding: core c owns head h = c for ALL 4 batches. Head h covers output
features [h*128, (h+1)*128), so batchnorm statistics over (batch, seq)
are fully local to the core: no cross-core collective at all.

All device compute is feature-major so every matmul contracts over the
partition dim with zero on-chip transposes:

  QT[u,t] = Wq_h @ query[b].T     fp8 DoubleRow (K=256/pass), descaled
  KT[u,t] = Wk_h @ keys[b].T      fp8 DoubleRow from on-chip-cast keys
  V[t,u]  = keys[b] @ Wv_h.T      bf16 (N=128 matmuls, FWL), stored fp8
  ST[k,q] = KT.T-contract QT      bf16 (K=128: DoubleRow not applicable)
  PT      = exp(ST)               ACT, PSUM->SBUF, fp8 (scores in [-2,2])
  OT[u,q] = sum_k V[k,u]*PT[k,q]  fp8 DoubleRow
  r[q]    = sum_k PT[k,q]         fp8 DoubleRow ones-matmuls (f32 acc)
  o_res   = OT/r + query[b].T     f32 residual
  batchnorm over (b,s): bn_stats per chunk, local bn_aggr, affine.

fp8 scaling: weights x32 on host (unit std fits e4m3's 3-bit mantissa);
the 1/32 score scale plus the x32x32 descale folds into the QT/KT
copy-out factor sq = sk = 1/sqrt(32768). Accuracy: scale-rel absmax
9.7e-3, mean rel 4.8e-3 (bf16-everything baseline was 7.9e-4).

Trace-verified facts this schedule is built on:
- DoubleRow fp8 matmuls issue every ~216ns warm (true 2x over bf16);
  requires [128, 2, X] slices of both operands, fp8e4, K=256/pass.
- Per-DMA-queue throughput is DESCRIPTOR-rate-bound (~23 desc/us, one
  descriptor per partition per DMA), so layouts keep [P] outermost with
  4-16KB contiguous per partition, keys ship ONCE as bf16 (the fp8 copy
  for the K-projection is cast on-chip by the DVE), and the three
  streams ride gpsimd/sync/scalar queues round-robin.
- exp on ACT is 1.11ns/elem ([128,2,512] pairs, 8.9us per 512-q chunk)
  = 142us total; the DVE rowsum tree would cost ~94us (fp8 reads are
  1 elem/cyc), so the rowsum rides the tensor engine as DoubleRow
  ones-matmuls instead (exact f32 accumulation, frees the DVE).
- The per-chunk emission interleaves AV+rowsum of chunk N-1 and
  projection work for later batches between the score matmuls of chunk
  N (engine FIFOs = program order!), with flush() fences so nothing is
  read before its writer is emitted; the finalize unit pops at the 7th
  score-pair so reciprocal->normalize->residual->bn_stats never blocks
  the tensor FIFO.
- gpsimd tensor ops are ~5x slower than DVE and gpsimd has NO PSUM
  port; keep it to DMA issue only.
- HAM: PE idle >3.4us re-throttles the clock to 1.2GHz; 20 warmup
  matmuls at t=0 + gap-free steady state keep it at 2.4GHz from ~50us.

Known remaining losses (~55us vs the ~185us tensor-work floor):
startup ~25us (batch-0's 6MB must land through cold DMA queues before
the first scores; first exp ~38us), tail ~20us (last-chunk exp drain +
bn finale + 4MB writeout + end barrier), HAM-cold inflation early.
Tried and REVERTED (net-negative, ~±5us run-to-run noise): batch-0
Q-before-K emission with per-half DMAs (284us!), half-batch kt16 tiles
with 16KB descriptors (249us), exp per single k-tile (ACT overhead
256ns/instr makes 16 singles cost more than 8 pairs).
"""
import sys

sys.path.insert(0, "/opt/trn_rl_repo")

from collections import deque

import numpy as np

import concourse.bass as bass
import concourse.tile as tile
from concourse import bacc, mybir
from concourse.bass_utils import run_bass_kernel_spmd

F32 = mybir.dt.float32
BF16 = mybir.dt.bfloat16
FP8 = mybir.dt.float8e4
AF = mybir.ActivationFunctionType
PM_DR = mybir.MatmulPerfMode.DoubleRow
ALU = mybir.AluOpType
NPBF16 = mybir.dt.np(BF16)
NPFP8 = mybir.dt.np(FP8)

B, S, D, H = 4, 2048, 1024, 8
DH = 128
P = 128
TC = 4                # 512-token chunks per sequence
TCW = 512
DT = 8                # 128-wide d-tiles in D
DP = 4                # d-tile pairs (DoubleRow K=256)
KT_N = 16             # 128-wide k-tiles per sequence
EPS = 1e-5
WSCALE = 32.0
SQK = 1.0 / np.sqrt(32768.0)   # QT/KT copy-out descale; sq*sk*D = 1/32


def _build():
    nc = bacc.Bacc(num_swdge_queues=1)
    qt8 = nc.declare_dram_parameter(
        "qt8", [B, P, TC, DP, 2, TCW], FP8, isOutput=False)
    kt16 = nc.declare_dram_parameter(
        "kt16", [B, P, TC, DT, TCW], BF16, isOutput=False)
    wqk8 = nc.declare_dram_parameter(
        "wqk8", [P, 2, DP, 2, DH], FP8, isOutput=False)
    wv16 = nc.declare_dram_parameter("wv16", [P, DT, DH], BF16, isOutput=False)
    qres = nc.declare_dram_parameter("qres", [B, P, S], F32, isOutput=False)
    gamma = nc.declare_dram_parameter("gamma", [P, 1], F32, isOutput=False)
    beta = nc.declare_dram_parameter("beta", [P, 1], F32, isOutput=False)
    out = nc.declare_dram_parameter("out", [P, B, S], F32, isOutput=True)

    with tile.TileContext(nc) as tc:
        with (
            tc.tile_pool(name="persist", bufs=1) as persist,
            tc.tile_pool(name="xq8", bufs=2) as xq8p,       # per-batch tiles
            tc.tile_pool(name="xk16", bufs=6) as xk16p,     # per-tc tiles
            tc.tile_pool(name="kf8", bufs=6) as kf8p,       # cast scratch
            tc.tile_pool(name="pt", bufs=2) as ptp,
            tc.tile_pool(name="rb", bufs=2) as rbp,
            tc.tile_pool(name="otmp", bufs=2) as otmpp,
            tc.tile_pool(name="ppsum", bufs=2, space="PSUM") as ppsum,
            tc.tile_pool(name="spsum", bufs=2, space="PSUM") as spsum,
            tc.tile_pool(name="opsum", bufs=2, space="PSUM") as opsum,
        ):
            # ---- persistent SBUF ----
            QT = persist.tile([P, B, S], BF16)            # (dh, b, q) 16KB/p
            KT = persist.tile([P, B, KT_N, P], BF16)      # (dh, b, kt, k) 16KB/p
            V8 = persist.tile([P, B, KT_N, DH], FP8)      # (t128, b, kt, u) 8KB/p
            o_res = persist.tile([P, B, S], F32)          # 32KB/p
            bstat = persist.tile([P, B * TC, nc.vector.BN_STATS_DIM], F32)
            wqk_s = persist.tile([P, 2, DP, 2, DH], FP8)
            wk_s = wqk_s[:, 0]
            wq_s = wqk_s[:, 1]
            wv_s = persist.tile([P, DT, DH], BF16)
            gam = persist.tile([P, 1], F32)
            bet = persist.tile([P, 1], F32)
            ones_b = persist.tile([P, P], BF16)
            ones8 = persist.tile([P, 2, P], FP8)
            eps_t = persist.tile([P, 1], F32)
            warm = persist.tile([P, 1], F32)
            mv = persist.tile([P, 2], F32)
            stdt = persist.tile([P, 1], F32)
            rstd = persist.tile([P, 1], F32)
            scl = persist.tile([P, 1], F32)
            shf = persist.tile([P, 1], F32)

            # ---- preamble ----
            nc.vector.memset(eps_t[:], float(EPS))
            nc.vector.memset(ones_b[:], 1.0)
            nc.vector.memset(ones8[:], 1.0)

            xtiles = {}   # streamed tiles keyed by (kind, b[, tc])

            # batch-0 keys stream first (K-proj gates everything),
            # spread across all three DMA queues
            qeng = [nc.gpsimd, nc.sync, nc.scalar, nc.gpsimd]
            nc.scalar.dma_start(wqk_s[:], wqk8[:])
            k0 = []
            for tci in range(TC):
                t = xk16p.tile([P, DT, TCW], BF16, tag="xk16", name="t")
                xtiles[("k16", 0, tci)] = (t, None)
                k0.append(t)
            nc.gpsimd.dma_start(k0[0][:], kt16[0, :, 0])
            nc.sync.dma_start(k0[1][:], kt16[0, :, 1])
            nc.scalar.dma_start(k0[2][:], kt16[0, :, 2])
            nc.gpsimd.dma_start(k0[3][:], kt16[0, :, 3])
            t0q = xq8p.tile([P, TC, DP, 2, TCW], FP8, tag="xq", name="t0q")
            nc.sync.dma_start(t0q[:], qt8[0])
            xtiles[("q8", 0)] = t0q
            nc.scalar.dma_start(wv_s[:], wv16[:])
            nc.gpsimd.dma_start(o_res[:, 0, :], qres[0])
            nc.scalar.dma_start(gam[:], gamma[:])
            nc.scalar.dma_start(bet[:], beta[:])
            nc.scalar.activation(out=warm[:], in_=eps_t[:], func=AF.Exp)

            # PE warmup: pull the HAM clock gate to 8/8 before real work
            wps = ppsum.tile([P, TCW], F32, tag="pp", name="wps")
            for _ in range(20):
                nc.tensor.matmul(
                    wps[:, 0:P], ones_b[:], ones_b[:],
                    start=True, stop=True, skip_group_check=True,
                )

            def dma_batch(b):
                for tci in range(TC):
                    t = xk16p.tile([P, DT, TCW], BF16, tag="xk16")
                    qeng[(b + tci) % 3].dma_start(t[:], kt16[b, :, tci])
                    xtiles[("k16", b, tci)] = (t, None)
                t = xq8p.tile([P, TC, DP, 2, TCW], FP8, tag="xq")
                nc.sync.dma_start(t[:], qt8[b])
                xtiles[("q8", b)] = t
                nc.scalar.dma_start(o_res[:, b, :], qres[b])

            # ---- tensor-work units (each ~0.3-0.7us of PE time) ----
            def kproj_units(b):
                units = []
                for tci in range(TC):
                    ps = [None]

                    def u1(b=b, tci=tci, ps=ps):
                        ps[0] = ppsum.tile([P, TC, P], F32, tag="pp", name="pk")
                        xt, _ = xtiles[("k16", b, tci)]
                        for dp in range(2):
                            kf = kf8p.tile([P, 2, TCW], FP8, tag="kf", name="kf")
                            nc.vector.tensor_copy(
                                kf[:], xt[:, bass.ts(dp, 2), :]
                            )
                            nc.tensor.matmul(
                                ps[0][:], wk_s[:, dp], kf[:],
                                start=(dp == 0), stop=False,
                                perf_mode=PM_DR, skip_group_check=True,
                            )

                    def u2(b=b, tci=tci, ps=ps):
                        xt, _ = xtiles[("k16", b, tci)]
                        for dp in range(2, DP):
                            kf = kf8p.tile([P, 2, TCW], FP8, tag="kf", name="kf")
                            nc.vector.tensor_copy(
                                kf[:], xt[:, bass.ts(dp, 2), :]
                            )
                            nc.tensor.matmul(
                                ps[0][:], wk_s[:, dp], kf[:],
                                start=False, stop=(dp == DP - 1),
                                perf_mode=PM_DR, skip_group_check=True,
                            )
                        nc.vector.tensor_scalar(
                            KT[:, b, bass.ts(tci, TC), :], ps[0][:],
                            float(SQK), None, ALU.mult,
                        )

                    units += [u1, u2]
                return units

            def qproj_units(b, tci):
                ps = [None]

                def u1(b=b, tci=tci, ps=ps):
                    ps[0] = ppsum.tile([P, TCW], F32, tag="pp", name="pq")
                    xt = xtiles[("q8", b)]
                    for dp in range(2):
                        nc.tensor.matmul(
                            ps[0][:], wq_s[:, dp], xt[:, tci, dp],
                            start=(dp == 0), stop=False,
                            perf_mode=PM_DR, skip_group_check=True,
                        )

                def u2(b=b, tci=tci, ps=ps):
                    xt = xtiles[("q8", b)]
                    for dp in range(2, DP):
                        nc.tensor.matmul(
                            ps[0][:], wq_s[:, dp], xt[:, tci, dp],
                            start=False, stop=(dp == DP - 1),
                            perf_mode=PM_DR, skip_group_check=True,
                        )
                    nc.vector.tensor_scalar(
                        QT[:, b, bass.ts(tci, TCW)], ps[0][:],
                        float(SQK), None, ALU.mult,
                    )

                return [u1, u2]

            def vproj_units(b):
                units = []
                for tci in range(TC):
                    ps = [None]
                    for sub in range(4):

                        def u(b=b, tci=tci, sub=sub, ps=ps):
                            if sub == 0:
                                ps[0] = ppsum.tile([P, 4, DH], F32, tag="pp",
                                                   name="pv")
                            xt, _ = xtiles[("k16", b, tci)]
                            for d in range(DT):
                                nc.tensor.matmul(
                                    ps[0][:, sub, :],
                                    xt[:, d, bass.ts(sub, P)],
                                    wv_s[:, d, :],
                                    start=(d == 0), stop=(d == DT - 1),
                                    skip_group_check=True,
                                )
                            if sub == 3:
                                nc.vector.tensor_copy(
                                    V8[:, b, bass.ts(tci, 4), :], ps[0][:]
                                )

                        units.append(u)
                return units

            pending = {}       # key -> deque of unit callables
            order = deque()    # key pop order
            drain_q = deque()
            late_q = deque()

            def push(key, units):
                pending[key] = deque(units)
                order.append(key)

            def flush(key):
                q = pending.get(key)
                while q:
                    q.popleft()()

            def pop_fill(n):
                for _ in range(n):
                    if drain_q:
                        drain_q.popleft()()
                        continue
                    while order and not pending.get(order[0]):
                        order.popleft()
                    if order:
                        pending[order[0]].popleft()()

            prev = {}

            def make_drain(b, q_i, PT, ps_o, ps_r):
                """AV + rowsum of chunk (b, q_i): 4 units x (2+2) DR MMs."""
                units = []
                for g in range(4):

                    def uav(g=g, b=b, PT=PT, ps_o=ps_o, ps_r=ps_r):
                        for kp in (2 * g, 2 * g + 1):
                            nc.tensor.matmul(
                                ps_o[:],
                                V8[:, b, bass.ts(kp, 2), :],
                                PT[:, bass.ts(kp, 2), :],
                                start=(kp == 0), stop=(kp == KT_N // 2 - 1),
                                perf_mode=PM_DR, skip_group_check=True,
                            )
                            nc.tensor.matmul(
                                ps_r[:],
                                ones8[:],
                                PT[:, bass.ts(kp, 2), :],
                                start=(kp == 0), stop=(kp == KT_N // 2 - 1),
                                perf_mode=PM_DR, skip_group_check=True,
                            )

                    units.append(uav)
                return units

            def make_fin(b, q_i, ps_o, ps_r):
                """1/r + attention normalize + residual add + bn_stats."""

                def ufin(b=b, q_i=q_i, ps_o=ps_o, ps_r=ps_r):
                    rb = rbp.tile([P, TCW], F32, tag="rb")
                    nc.vector.reciprocal_approx_fast(out=rb[:], in_=ps_r[:])
                    otmp = otmpp.tile([P, TCW], F32, tag="ot")
                    nc.vector.tensor_tensor(otmp[:], ps_o[:], rb[:], ALU.mult)
                    dst = o_res[:, b, bass.ts(q_i, TCW)]
                    nc.vector.tensor_add(dst, dst, otmp[:])
                    nc.vector.bn_stats(out=bstat[:, b * TC + q_i, :], in_=dst)

                return ufin

            def emit_chunk(b, q_i):
                # correctness fences: everything this chunk's matmuls read
                # must already be emitted (program order defines deps)
                flush(("k", b))
                flush(("q", b, q_i))
                PT = ptp.tile([P, KT_N, TCW], FP8, tag="pt")
                if prev:
                    flush(("v", prev["b"]))   # AV drain needs V tiles
                    drain_q.extend(
                        make_drain(prev["b"], prev["q_i"], prev["PT"],
                                   prev["ps_o"], prev["ps_r"])
                    )
                    late_q.append(
                        make_fin(prev["b"], prev["q_i"], prev["ps_o"],
                                 prev["ps_r"])
                    )
                ps_o = opsum.tile([P, TCW], F32, tag="op", name="ps_o")
                ps_r = opsum.tile([P, TCW], F32, tag="op", name="ps_r")
                for kp in range(KT_N // 2):
                    ps_s = spsum.tile([P, 2, TCW], F32, tag="sp")
                    for j in range(2):
                        nc.tensor.matmul(
                            ps_s[:, j, :],
                            KT[:, b, 2 * kp + j, :],
                            QT[:, b, bass.ts(q_i, TCW)],
                            start=True, stop=True, skip_group_check=True,
                        )
                    nc.scalar.activation(
                        out=PT[:, bass.ts(kp, 2), :], in_=ps_s[:], func=AF.Exp
                    )
                    if kp == 6 and late_q:
                        late_q.popleft()()
                        pop_fill(1)
                    else:
                        pop_fill(2)
                prev.clear()
                prev.update({"b": b, "q_i": q_i, "PT": PT, "ps_o": ps_o,
                             "ps_r": ps_r})

            # ---- emission ----
            for u in kproj_units(0):
                u()
            for u in qproj_units(0, 0):
                u()

            for b in range(B):
                for q_i in range(TC):
                    if b == 0 and q_i == 0:
                        push(("v", 0), vproj_units(0))
                    if q_i == 0 and b < B - 1:
                        dma_batch(b + 1)
                    if q_i == 1 and b < B - 1:
                        push(("k", b + 1), kproj_units(b + 1))
                    if q_i == 2 and b < B - 1:
                        push(("q", b + 1, 0), qproj_units(b + 1, 0))
                        push(("v", b + 1), vproj_units(b + 1))
                    if q_i < TC - 1:
                        push(("q", b, q_i + 1), qproj_units(b, q_i + 1))
                    emit_chunk(b, q_i)

            # drain the last chunk + any remaining stragglers
            flush(("v", prev["b"]))
            drain_q.extend(
                make_drain(prev["b"], prev["q_i"], prev["PT"], prev["ps_o"],
                           prev["ps_r"])
            )
            late_q.append(
                make_fin(prev["b"], prev["q_i"], prev["ps_o"], prev["ps_r"])
            )
            nc.scalar.activation(out=warm[:], in_=eps_t[:], func=AF.Sqrt)
            while drain_q or any(pending.get(k) for k in list(order)):
                pop_fill(1)
            while late_q:
                late_q.popleft()()

            # ---- batchnorm finale (fully local) ----
            nc.vector.bn_aggr(out=mv[:], in_=bstat[:])
            nc.scalar.activation(
                out=stdt[:], in_=mv[:, 1:2], func=AF.Sqrt, bias=eps_t[:]
            )
            nc.vector.reciprocal(out=rstd[:], in_=stdt[:])
            nc.vector.tensor_mul(scl[:], gam[:], rstd[:])
            nc.vector.tensor_mul(shf[:], mv[:, 0:1], scl[:])
            nc.vector.tensor_sub(shf[:], bet[:], shf[:])
            for b in range(B):
                for half in range(2):
                    sl = bass.ts(half, S // 2)
                    src = o_res[:, b, sl]
                    if half == 0:
                        nc.vector.tensor_scalar(
                            src, src, scl[:], shf[:], ALU.mult, ALU.add
                        )
                    else:
                        nc.scalar.activation(
                            out=src, in_=src, func=AF.Identity,
                            bias=shf[:], scale=scl[:],
                        )
                weng = nc.sync if b % 2 == 0 else nc.scalar
                weng.dma_start(out[:, b, :], o_res[:, b, :])

    nc.finalize()
    return nc


_NC = None


def _get_nc():
    global _NC
    if _NC is None:
        _NC = _build()
    return _NC


def _make_in_maps(query, keys, Wq, Wk, Wv, gamma, beta):
    query = np.asarray(query, dtype=np.float32)
    keys = np.asarray(keys, dtype=np.float32)
    Wq = np.asarray(Wq, dtype=np.float32)
    Wk = np.asarray(Wk, dtype=np.float32)
    Wv = np.asarray(Wv, dtype=np.float32)
    gamma = np.asarray(gamma, dtype=np.float32)
    beta = np.asarray(beta, dtype=np.float32)

    qT = np.ascontiguousarray(query.transpose(0, 2, 1))   # (B, D, S)
    kT = np.ascontiguousarray(keys.transpose(0, 2, 1))

    # (B, D, S) -> [B, P, TC, DP, 2, TCW] fp8 (P outermost per batch)
    v = qT.reshape(B, DP, 2, P, TC, TCW).transpose(0, 3, 4, 1, 2, 5)
    qt8 = np.ascontiguousarray(v.astype(NPFP8))

    # (B, D, S) -> [B, P, TC, DT, TCW] bf16
    v = kT.reshape(B, DT, P, TC, TCW).transpose(0, 2, 3, 1, 4)
    kt16 = np.ascontiguousarray(v.astype(NPBF16))

    in_maps = []
    for c in range(8):
        rows = slice(DH * c, DH * (c + 1))

        def packw8(w):  # rows of W -> [P, DP, 2, DH] fp8, scaled x32
            wt = np.ascontiguousarray(w[rows].T * WSCALE)   # (D, 128)
            v = wt.reshape(DP, 2, P, DH).transpose(2, 0, 1, 3)
            return np.ascontiguousarray(v.astype(NPFP8))

        wqk8 = np.ascontiguousarray(
            np.stack([packw8(Wk), packw8(Wq)], axis=1)
        )  # [P, 2, DP, 2, DH]

        wv_t = np.ascontiguousarray(Wv[rows].T)             # (D, 128)
        in_maps.append(
            {
                "qt8": qt8,
                "kt16": kt16,
                "wqk8": wqk8,
                "wv16": np.ascontiguousarray(
                    wv_t.reshape(DT, P, DH).transpose(1, 0, 2).astype(NPBF16)
                ),
                "qres": np.ascontiguousarray(
                    query[:, :, rows].transpose(0, 2, 1)
                ),  # (B, 128, S)
                "gamma": np.ascontiguousarray(gamma[rows].reshape(P, 1)),
                "beta": np.ascontiguousarray(beta[rows].reshape(P, 1)),
            }
        )
    return in_maps


def _run(in_maps, trace=False, **kw):
    nc = _get_nc()
    return run_bass_kernel_spmd(
        nc, in_maps, core_ids=list(range(8)), trace=trace, **kw
    )


def kernel(query, keys, Wq, Wk, Wv, gamma, beta):
    in_maps = _make_in_maps(query, keys, Wq, Wk, Wv, gamma, beta)
    res = _run(in_maps)
    output = np.empty((B, S, D), dtype=np.float32)
    for c in range(8):
        oc = res.results[c]["out"]                    # (128, B, S)
        output[:, :, DH * c : DH * (c + 1)] = oc.transpose(1, 2, 0)
    return output


# revision 18
# speedup vs baseline: 1.0290x; 1.0071x over previous
"""Multi-head attention + residual + batchnorm on 8 trn2 NeuronCores.
Measured: 238974 ns (baseline two-phase (b,head-group) kernel: 303878).

Sharding: core c owns head h = c for ALL 4 batches. Head h covers output
features [h*128, (h+1)*128), so batchnorm statistics over (batch, seq)
are fully local to the core: no cross-core collective at all.

All device compute is feature-major so every matmul contracts over the
partition dim with zero on-chip transposes:

  QT[u,t] = Wq_h @ query[b].T     fp8 DoubleRow (K=256/pass), descaled
  KT[u,t] = Wk_h @ keys[b].T      fp8 DoubleRow from on-chip-cast keys
  V[t,u]  = keys[b] @ Wv_h.T      bf16 (N=128 matmuls, FWL), stored fp8
  ST[k,q] = KT.T-contract QT      bf16 (K=128: DoubleRow not applicable)
  PT      = exp(ST)               ACT, PSUM->SBUF, fp8 (scores in [-2,2])
  OT[u,q] = sum_k V[k,u]*PT[k,q]  fp8 DoubleRow
  r[q]    = sum_k PT[k,q]         fp8 DoubleRow ones-matmuls (f32 acc)
  o_res   = OT/r + query[b].T     f32 residual
  batchnorm over (b,s): bn_stats per chunk, local bn_aggr, affine.

fp8 scaling: weights x32 on host (unit std fits e4m3's 3-bit mantissa);
the 1/32 score scale plus the x32x32 descale folds into the QT/KT
copy-out factor sq = sk = 1/sqrt(32768). Accuracy: scale-rel absmax
9.7e-3, mean rel 4.8e-3 (bf16-everything baseline was 7.9e-4).

Trace-verified facts this schedule is built on:
- DoubleRow fp8 matmuls issue every ~216ns warm (true 2x over bf16);
  requires [128, 2, X] slices of both operands, fp8e4, K=256/pass.
- Per-DMA-queue throughput is DESCRIPTOR-rate-bound (~23 desc/us, one
  descriptor per partition per DMA), so layouts keep [P] outermost with
  4-16KB contiguous per partition, keys ship ONCE as bf16 (the fp8 copy
  for the K-projection is cast on-chip by the DVE), and the three
  streams ride gpsimd/sync/scalar queues round-robin.
- exp on ACT is 1.11ns/elem ([128,2,512] pairs, 8.9us per 512-q chunk)
  = 142us total; the DVE rowsum tree would cost ~94us (fp8 reads are
  1 elem/cyc), so the rowsum rides the tensor engine as DoubleRow
  ones-matmuls instead (exact f32 accumulation, frees the DVE).
- The per-chunk emission interleaves AV+rowsum of chunk N-1 and
  projection work for later batches between the score matmuls of chunk
  N (engine FIFOs = program order!), with flush() fences so nothing is
  read before its writer is emitted; the finalize unit pops at the 7th
  score-pair so reciprocal->normalize->residual->bn_stats never blocks
  the tensor FIFO.
- gpsimd tensor ops are ~5x slower than DVE and gpsimd has NO PSUM
  port; keep it to DMA issue only.
- HAM: PE idle >3.4us re-throttles the clock to 1.2GHz; 20 warmup
  matmuls at t=0 + gap-free steady state keep it at 2.4GHz from ~50us.

Known remaining losses (~55us vs the ~185us tensor-work floor):
startup ~25us (batch-0's 6MB must land through cold DMA queues before
the first scores; first exp ~38us), tail ~20us (last-chunk exp drain +
bn finale + 4MB writeout + end barrier), HAM-cold inflation early.
Tried and REVERTED (net-negative, ~±5us run-to-run noise): batch-0
Q-before-K emission with per-half DMAs (284us!), half-batch kt16 tiles
with 16KB descriptors (249us), exp per single k-tile (ACT overhead
256ns/instr makes 16 singles cost more than 8 pairs), merged wq8+wk8
into one 2KB/p DMA + whole-batch batch-0 qt8 (first exp moved 38->53us,
244us: big descriptors do NOT beat sliced ones; the 8KB/desc ~175GB/s
queue model breaks down at startup, arrival order >> descriptor math).
Config measured 5x: 237.4/237.8/238.9/241.4/244.5 (pick = this file).
"""
import sys

sys.path.insert(0, "/opt/trn_rl_repo")

from collections import deque

import numpy as np

import concourse.bass as bass
import concourse.tile as tile
from concourse import bacc, mybir
from concourse.bass_utils import run_bass_kernel_spmd

F32 = mybir.dt.float32
BF16 = mybir.dt.bfloat16
FP8 = mybir.dt.float8e4
AF = mybir.ActivationFunctionType
PM_DR = mybir.MatmulPerfMode.DoubleRow
ALU = mybir.AluOpType
NPBF16 = mybir.dt.np(BF16)
NPFP8 = mybir.dt.np(FP8)

B, S, D, H = 4, 2048, 1024, 8
DH = 128
P = 128
TC = 4                # 512-token chunks per sequence
TCW = 512
DT = 8                # 128-wide d-tiles in D
DP = 4                # d-tile pairs (DoubleRow K=256)
KT_N = 16             # 128-wide k-tiles per sequence
EPS = 1e-5
WSCALE = 32.0
SQK = 1.0 / np.sqrt(32768.0)   # QT/KT copy-out descale; sq*sk*D = 1/32


def _build():
    nc = bacc.Bacc(num_swdge_queues=1)
    qt8 = nc.declare_dram_parameter(
        "qt8", [B, P, TC, DP, 2, TCW], FP8, isOutput=False)
    kt16 = nc.declare_dram_parameter(
        "kt16", [B, P, TC, DT, TCW], BF16, isOutput=False)
    wq8 = nc.declare_dram_parameter("wq8", [P, DP, 2, DH], FP8, isOutput=False)
    wk8 = nc.declare_dram_parameter("wk8", [P, DP, 2, DH], FP8, isOutput=False)
    wv16 = nc.declare_dram_parameter("wv16", [P, DT, DH], BF16, isOutput=False)
    qres = nc.declare_dram_parameter("qres", [B, P, S], F32, isOutput=False)
    gamma = nc.declare_dram_parameter("gamma", [P, 1], F32, isOutput=False)
    beta = nc.declare_dram_parameter("beta", [P, 1], F32, isOutput=False)
    out = nc.declare_dram_parameter("out", [P, B, S], F32, isOutput=True)

    with tile.TileContext(nc) as tc:
        with (
            tc.tile_pool(name="persist", bufs=1) as persist,
            tc.tile_pool(name="xq8", bufs=2) as xq8p,       # per-batch tiles
            tc.tile_pool(name="xk16", bufs=6) as xk16p,     # per-tc tiles
            tc.tile_pool(name="kf8", bufs=6) as kf8p,       # cast scratch
            tc.tile_pool(name="pt", bufs=2) as ptp,
            tc.tile_pool(name="rb", bufs=2) as rbp,
            tc.tile_pool(name="otmp", bufs=2) as otmpp,
            tc.tile_pool(name="ppsum", bufs=2, space="PSUM") as ppsum,
            tc.tile_pool(name="spsum", bufs=2, space="PSUM") as spsum,
            tc.tile_pool(name="opsum", bufs=2, space="PSUM") as opsum,
        ):
            # ---- persistent SBUF ----
            QT = persist.tile([P, B, S], BF16)            # (dh, b, q) 16KB/p
            KT = persist.tile([P, B, KT_N, P], BF16)      # (dh, b, kt, k) 16KB/p
            V8 = persist.tile([P, B, KT_N, DH], FP8)      # (t128, b, kt, u) 8KB/p
            o_res = persist.tile([P, B, S], F32)          # 32KB/p
            bstat = persist.tile([P, B * TC, nc.vector.BN_STATS_DIM], F32)
            wq_s = persist.tile([P, DP, 2, DH], FP8)
            wk_s = persist.tile([P, DP, 2, DH], FP8)
            wv_s = persist.tile([P, DT, DH], BF16)
            gam = persist.tile([P, 1], F32)
            bet = persist.tile([P, 1], F32)
            ones_b = persist.tile([P, P], BF16)
            ones8 = persist.tile([P, 2, P], FP8)
            eps_t = persist.tile([P, 1], F32)
            warm = persist.tile([P, 1], F32)
            mv = persist.tile([P, 2], F32)
            stdt = persist.tile([P, 1], F32)
            rstd = persist.tile([P, 1], F32)
            scl = persist.tile([P, 1], F32)
            shf = persist.tile([P, 1], F32)

            # ---- preamble ----
            nc.vector.memset(eps_t[:], float(EPS))
            nc.vector.memset(ones_b[:], 1.0)
            nc.vector.memset(ones8[:], 1.0)

            xtiles = {}   # streamed tiles keyed by (kind, b[, tc])

            # batch-0 keys stream first (K-proj gates everything),
            # spread across all three DMA queues
            qeng = [nc.gpsimd, nc.sync, nc.scalar, nc.gpsimd]
            nc.scalar.dma_start(wk_s[:], wk8[:])
            for tci in range(TC):
                t = xk16p.tile([P, DT, TCW], BF16, tag="xk16", name="t")
                qeng[tci].dma_start(t[:], kt16[0, :, tci])
                xtiles[("k16", 0, tci)] = (t, None)
            nc.scalar.dma_start(wq_s[:], wq8[:])
            t0q = xq8p.tile([P, TC, DP, 2, TCW], FP8, tag="xq", name="t0q")
            nc.sync.dma_start(t0q[:, 0], qt8[0, :, 0])
            nc.sync.dma_start(t0q[:, 1], qt8[0, :, 1])
            nc.gpsimd.dma_start(t0q[:, 2], qt8[0, :, 2])
            nc.gpsimd.dma_start(t0q[:, 3], qt8[0, :, 3])
            xtiles[("q8", 0)] = t0q
            nc.scalar.dma_start(wv_s[:], wv16[:])
            nc.gpsimd.dma_start(o_res[:, 0, :], qres[0])
            nc.scalar.dma_start(gam[:], gamma[:])
            nc.scalar.dma_start(bet[:], beta[:])
            nc.scalar.activation(out=warm[:], in_=eps_t[:], func=AF.Exp)

            # PE warmup: pull the HAM clock gate to 8/8 before real work
            wps = ppsum.tile([P, TCW], F32, tag="pp", name="wps")
            for _ in range(20):
                nc.tensor.matmul(
                    wps[:, 0:P], ones_b[:], ones_b[:],
                    start=True, stop=True, skip_group_check=True,
                )

            def dma_batch(b):
                for tci in range(TC):
                    t = xk16p.tile([P, DT, TCW], BF16, tag="xk16")
                    qeng[(b + tci) % 3].dma_start(t[:], kt16[b, :, tci])
                    xtiles[("k16", b, tci)] = (t, None)
                t = xq8p.tile([P, TC, DP, 2, TCW], FP8, tag="xq")
                nc.sync.dma_start(t[:], qt8[b])
                xtiles[("q8", b)] = t
                nc.scalar.dma_start(o_res[:, b, :], qres[b])

            # ---- tensor-work units (each ~0.3-0.7us of PE time) ----
            def kproj_units(b):
                units = []
                for tci in range(TC):
                    ps = [None]

                    def u1(b=b, tci=tci, ps=ps):
                        ps[0] = ppsum.tile([P, TC, P], F32, tag="pp", name="pk")
                        xt, _ = xtiles[("k16", b, tci)]
                        for dp in range(2):
                            kf = kf8p.tile([P, 2, TCW], FP8, tag="kf", name="kf")
                            nc.vector.tensor_copy(
                                kf[:], xt[:, bass.ts(dp, 2), :]
                            )
                            nc.tensor.matmul(
                                ps[0][:], wk_s[:, dp], kf[:],
                                start=(dp == 0), stop=False,
                                perf_mode=PM_DR, skip_group_check=True,
                            )

                    def u2(b=b, tci=tci, ps=ps):
                        xt, _ = xtiles[("k16", b, tci)]
                        for dp in range(2, DP):
                            kf = kf8p.tile([P, 2, TCW], FP8, tag="kf", name="kf")
                            nc.vector.tensor_copy(
                                kf[:], xt[:, bass.ts(dp, 2), :]
                            )
                            nc.tensor.matmul(
                                ps[0][:], wk_s[:, dp], kf[:],
                                start=False, stop=(dp == DP - 1),
                                perf_mode=PM_DR, skip_group_check=True,
                            )
                        nc.vector.tensor_scalar(
                            KT[:, b, bass.ts(tci, TC), :], ps[0][:],
                            float(SQK), None, ALU.mult,
                        )

                    units += [u1, u2]
                return units

            def qproj_units(b, tci):
                ps = [None]

                def u1(b=b, tci=tci, ps=ps):
                    ps[0] = ppsum.tile([P, TCW], F32, tag="pp", name="pq")
                    xt = xtiles[("q8", b)]
                    for dp in range(2):
                        nc.tensor.matmul(
                            ps[0][:], wq_s[:, dp], xt[:, tci, dp],
                            start=(dp == 0), stop=False,
                            perf_mode=PM_DR, skip_group_check=True,
                        )

                def u2(b=b, tci=tci, ps=ps):
                    xt = xtiles[("q8", b)]
                    for dp in range(2, DP):
                        nc.tensor.matmul(
                            ps[0][:], wq_s[:, dp], xt[:, tci, dp],
                            start=False, stop=(dp == DP - 1),
                            perf_mode=PM_DR, skip_group_check=True,
                        )
                    nc.vector.tensor_scalar(
                        QT[:, b, bass.ts(tci, TCW)], ps[0][:],
                        float(SQK), None, ALU.mult,
                    )

                return [u1, u2]

            def vproj_units(b):
                units = []
                for tci in range(TC):
                    ps = [None]
                    for sub in range(4):

                        def u(b=b, tci=tci, sub=sub, ps=ps):
                            if sub == 0:
                                ps[0] = ppsum.tile([P, 4, DH], F32, tag="pp",
                                                   name="pv")
                            xt, _ = xtiles[("k16", b, tci)]
                            for d in range(DT):
                                nc.tensor.matmul(
                                    ps[0][:, sub, :],
                                    xt[:, d, bass.ts(sub, P)],
                                    wv_s[:, d, :],
                                    start=(d == 0), stop=(d == DT - 1),
                                    skip_group_check=True,
                                )
                            if sub == 3:
                                nc.vector.tensor_copy(
                                    V8[:, b, bass.ts(tci, 4), :], ps[0][:]
                                )

                        units.append(u)
                return units

            pending = {}       # key -> deque of unit callables
            order = deque()    # key pop order
            drain_q = deque()
            late_q = deque()

            def push(key, units):
                pending[key] = deque(units)
                order.append(key)

            def flush(key):
                q = pending.get(key)
                while q:
                    q.popleft()()

            def pop_fill(n):
                for _ in range(n):
                    if drain_q:
                        drain_q.popleft()()
                        continue
                    while order and not pending.get(order[0]):
                        order.popleft()
                    if order:
                        pending[order[0]].popleft()()

            prev = {}

            def make_drain(b, q_i, PT, ps_o, ps_r):
                """AV + rowsum of chunk (b, q_i): 4 units x (2+2) DR MMs."""
                units = []
                for g in range(4):

                    def uav(g=g, b=b, PT=PT, ps_o=ps_o, ps_r=ps_r):
                        for kp in (2 * g, 2 * g + 1):
                            nc.tensor.matmul(
                                ps_o[:],
                                V8[:, b, bass.ts(kp, 2), :],
                                PT[:, bass.ts(kp, 2), :],
                                start=(kp == 0), stop=(kp == KT_N // 2 - 1),
                                perf_mode=PM_DR, skip_group_check=True,
                            )
                            nc.tensor.matmul(
                                ps_r[:],
                                ones8[:],
                                PT[:, bass.ts(kp, 2), :],
                                start=(kp == 0), stop=(kp == KT_N // 2 - 1),
                                perf_mode=PM_DR, skip_group_check=True,
                            )

                    units.append(uav)
                return units

            def make_fin(b, q_i, ps_o, ps_r):
                """1/r + attention normalize + residual add + bn_stats."""

                def ufin(b=b, q_i=q_i, ps_o=ps_o, ps_r=ps_r):
                    rb = rbp.tile([P, TCW], F32, tag="rb")
                    nc.vector.reciprocal_approx_fast(out=rb[:], in_=ps_r[:])
                    otmp = otmpp.tile([P, TCW], F32, tag="ot")
                    nc.vector.tensor_tensor(otmp[:], ps_o[:], rb[:], ALU.mult)
                    dst = o_res[:, b, bass.ts(q_i, TCW)]
                    nc.vector.tensor_add(dst, dst, otmp[:])
                    nc.vector.bn_stats(out=bstat[:, b * TC + q_i, :], in_=dst)

                return ufin

            def emit_chunk(b, q_i):
                # correctness fences: everything this chunk's matmuls read
                # must already be emitted (program order defines deps)
                flush(("k", b))
                flush(("q", b, q_i))
                PT = ptp.tile([P, KT_N, TCW], FP8, tag="pt")
                if prev:
                    flush(("v", prev["b"]))   # AV drain needs V tiles
                    drain_q.extend(
                        make_drain(prev["b"], prev["q_i"], prev["PT"],
                                   prev["ps_o"], prev["ps_r"])
                    )
                    late_q.append(
                        make_fin(prev["b"], prev["q_i"], prev["ps_o"],
                                 prev["ps_r"])
                    )
                ps_o = opsum.tile([P, TCW], F32, tag="op", name="ps_o")
                ps_r = opsum.tile([P, TCW], F32, tag="op", name="ps_r")
                for kp in range(KT_N // 2):
                    ps_s = spsum.tile([P, 2, TCW], F32, tag="sp")
                    for j in range(2):
                        nc.tensor.matmul(
                            ps_s[:, j, :],
                            KT[:, b, 2 * kp + j, :],
                            QT[:, b, bass.ts(q_i, TCW)],
                            start=True, stop=True, skip_group_check=True,
                        )
                    nc.scalar.activation(
                        out=PT[:, bass.ts(kp, 2), :], in_=ps_s[:], func=AF.Exp
                    )
                    if kp == 6 and late_q:
                        late_q.popleft()()
                        pop_fill(1)
                    else:
                        pop_fill(2)
                prev.clear()
                prev.update({"b": b, "q_i": q_i, "PT": PT, "ps_o": ps_o,
                             "ps_r": ps_r})

            # ---- emission ----
            for u in kproj_units(0):
                u()
            for u in qproj_units(0, 0):
                u()

            for b in range(B):
                for q_i in range(TC):
                    if b == 0 and q_i == 0:
                        push(("v", 0), vproj_units(0))
                    if q_i == 0 and b < B - 1:
                        dma_batch(b + 1)
                    if q_i == 1 and b < B - 1:
                        push(("k", b + 1), kproj_units(b + 1))
                    if q_i == 2 and b < B - 1:
                        push(("q", b + 1, 0), qproj_units(b + 1, 0))
                        push(("v", b + 1), vproj_units(b + 1))
                    if q_i < TC - 1:
                        push(("q", b, q_i + 1), qproj_units(b, q_i + 1))
                    emit_chunk(b, q_i)

            # drain the last chunk + any remaining stragglers
            flush(("v", prev["b"]))
            drain_q.extend(
                make_drain(prev["b"], prev["q_i"], prev["PT"], prev["ps_o"],
                           prev["ps_r"])
            )
            late_q.append(
                make_fin(prev["b"], prev["q_i"], prev["ps_o"], prev["ps_r"])
            )
            nc.scalar.activation(out=warm[:], in_=eps_t[:], func=AF.Sqrt)
            while drain_q or any(pending.get(k) for k in list(order)):
                pop_fill(1)
            while late_q:
                late_q.popleft()()

            # ---- batchnorm finale (fully local) ----
            nc.vector.bn_aggr(out=mv[:], in_=bstat[:])
            nc.scalar.activation(
                out=stdt[:], in_=mv[:, 1:2], func=AF.Sqrt, bias=eps_t[:]
            )
            nc.vector.reciprocal(out=rstd[:], in_=stdt[:])
            nc.vector.tensor_mul(scl[:], gam[:], rstd[:])
            nc.vector.tensor_mul(shf[:], mv[:, 0:1], scl[:])
            nc.vector.tensor_sub(shf[:], bet[:], shf[:])
            for b in range(B):
                for half in range(2):
                    sl = bass.ts(half, S // 2)
                    src = o_res[:, b, sl]
                    if half == 0:
                        nc.vector.tensor_scalar(
                            src, src, scl[:], shf[:], ALU.mult, ALU.add
                        )
                    else:
                        nc.scalar.activation(
                            out=src, in_=src, func=AF.Identity,
                            bias=shf[:], scale=scl[:],
                        )
                weng = nc.sync if b % 2 == 0 else nc.scalar
                weng.dma_start(out[:, b, :], o_res[:, b, :])

    nc.finalize()
    return nc


_NC = None


def _get_nc():
    global _NC
    if _NC is None:
        _NC = _build()
    return _NC


def _make_in_maps(query, keys, Wq, Wk, Wv, gamma, beta):
    query = np.asarray(query, dtype=np.float32)
    keys = np.asarray(keys, dtype=np.float32)
    Wq = np.asarray(Wq, dtype=np.float32)
    Wk = np.asarray(Wk, dtype=np.float32)
    Wv = np.asarray(Wv, dtype=np.float32)
    gamma = np.asarray(gamma, dtype=np.float32)
    beta = np.asarray(beta, dtype=np.float32)

    qT = np.ascontiguousarray(query.transpose(0, 2, 1))   # (B, D, S)
    kT = np.ascontiguousarray(keys.transpose(0, 2, 1))

    # (B, D, S) -> [B, P, TC, DP, 2, TCW] fp8 (P outermost per batch)
    v = qT.reshape(B, DP, 2, P, TC, TCW).transpose(0, 3, 4, 1, 2, 5)
    qt8 = np.ascontiguousarray(v.astype(NPFP8))

    # (B, D, S) -> [B, P, TC, DT, TCW] bf16
    v = kT.reshape(B, DT, P, TC, TCW).transpose(0, 2, 3, 1, 4)
    kt16 = np.ascontiguousarray(v.astype(NPBF16))

    in_maps = []
    for c in range(8):
        rows = slice(DH * c, DH * (c + 1))

        def packw8(w):  # rows of W -> [P, DP, 2, DH] fp8, scaled x32
            wt = np.ascontiguousarray(w[rows].T * WSCALE)   # (D, 128)
            v = wt.reshape(DP, 2, P, DH).transpose(2, 0, 1, 3)
            return np.ascontiguousarray(v.astype(NPFP8))


        wv_t = np.ascontiguousarray(Wv[rows].T)             # (D, 128)
        in_maps.append(
            {
                "qt8": qt8,
                "kt16": kt16,
                "wq8": packw8(Wq),
                "wk8": packw8(Wk),
                "wv16": np.ascontiguousarray(
                    wv_t.reshape(DT, P, DH).transpose(1, 0, 2).astype(NPBF16)
                ),
                "qres": np.ascontiguousarray(
                    query[:, :, rows].transpose(0, 2, 1)
                ),  # (B, 128, S)
                "gamma": np.ascontiguousarray(gamma[rows].reshape(P, 1)),
                "beta": np.ascontiguousarray(beta[rows].reshape(P, 1)),
            }
        )
    return in_maps


def _run(in_maps, trace=False, **kw):
    nc = _get_nc()
    return run_bass_kernel_spmd(
        nc, in_maps, core_ids=list(range(8)), trace=trace, **kw
    )


def kernel(query, keys, Wq, Wk, Wv, gamma, beta):
    in_maps = _make_in_maps(query, keys, Wq, Wk, Wv, gamma, beta)
    res = _run(in_maps)
    output = np.empty((B, S, D), dtype=np.float32)
    for c in range(8):
        oc = res.results[c]["out"]                    # (128, B, S)
        output[:, :, DH * c : DH * (c + 1)] = oc.transpose(1, 2, 0)
    return output
